# revision 29
# baseline (speedup 1.0000x reference)
"""Multi-head attention (B=4, S=2048, D=1024, H=16) on 8 Trainium2 NeuronCores.

Sharding: core c handles batch b=c//2 and head-group g=c%2 (8 heads = 512
features). Per core, transposed dataflow so every matmul contracts over the
SBUF partition dim; PSUM accumulation fp32 throughout.

Precision plan (rel-err budget 2e-2; errors concentrate at early queries
whose attention support is tiny, so those stay fp16):
  - Q/K/V projections: x and 16*w in fp8-e4m3, DoubleRow perf mode (pairs of
    128-feature contraction chunks per instruction -> 2x PE rate). Q/K tiles
    written fp16 (DVE tensor_scalar, scale 1/16 folds the weight scaling).
  - S = K^T Q: fp16 (d_k=64 contraction cannot use DoubleRow), trimmed at
    128-col granularity to the causal boundary rs. Even/odd heads issue
    bank-interleaved so they row-pack (row_grp 0/64) on the PE.
  - softmax: exp((s)/8) on ScalarE from PSUM, fp8 output; emission order
    e(kb0), o(kb0), e(kb1), o(kb1) so the S_e/S_o psum WAR recycle overlaps
    (S of next kb fills during the odd head's exp). Causal masking via a
    narrow 0/1 mask multiply over [rs0, rs0+256) only; AV reads are trimmed
    below rs0 so stale p-pool data is never consumed (no priming needed).
  - AV: Vhat in fp8 (16*V, ones column 1/4 for denominators), key-block
    PAIRS per DoubleRow instruction -> 2x rate. Exception: window 0,
    key blocks 0-1 run fp16 (single matmuls) with fp16 p and Vhat copies,
    protecting early queries from fp8 mantissa noise.
  - ctx fp16 (real scale; 1/16 folded into the reciprocal chain), output
    projection fp16, fp16 partial out. Host sums the two head-group
    partials + b_o in fp32.

All DRAM parameters are pre-rearranged on the host into their SBUF layouts
so every DMA moves contiguous per-partition rows (fast descriptors), and
the emission order front-loads what attention t=0 needs.
"""

import hashlib
import os
import shutil

import numpy as np

D_MODEL = 1024
N_HEADS = 16
D_K = 64
B = 4
S = 2048
N_CORES = 8
GS = 512            # per-core feature group (8 heads)
NT = GS // 128      # 4 feature tiles (head pairs) per core
NKB = S // 128      # 16 key blocks
W = 1024            # q window width
NW = S // W         # 2 windows

_prog_cache: dict = {}
_last_in_maps = None
_last_res = None


def _install_neff_cache():
    import concourse.bass2jax as b2j

    if getattr(b2j, "_ant_neff_cache_installed", False):
        return
    orig = b2j.compile_bir_kernel
    cache_dir = os.environ.get("BASS_NEFF_CACHE", "/tmp/bass_neff_cache")
    os.makedirs(cache_dir, exist_ok=True)

    def cached(bir_json, tmpdir, neff_name="file.neff"):
        data = bir_json if isinstance(bir_json, bytes) else bir_json.encode()
        h = hashlib.sha256(data).hexdigest()[:32]
        cpath = os.path.join(cache_dir, h + ".neff")
        dst = os.path.join(tmpdir, neff_name)
        if os.path.exists(cpath):
            shutil.copyfile(cpath, dst)
            return dst
        out = orig(bir_json, tmpdir, neff_name=neff_name)
        try:
            shutil.copyfile(out, cpath)
        except OSError:
            pass
        return out

    b2j.compile_bir_kernel = cached
    b2j._ant_neff_cache_installed = True


def _rel_start(kb: int, qh: int, mode: str) -> int:
    if mode == "full":
        return 0
    return max(0, kb * 128 - qh * W)


def _mm_pair(nc, DR, out, lhsT2, rhs2, start, stop):
    """DoubleRow pair matmul; MHA_NO_DR=1 splits into two plain matmuls."""
    if os.environ.get("MHA_NO_DR") != "1":
        nc.tensor.matmul(out, lhsT2(None), rhs2(None), start=start,
                         stop=stop, perf_mode=DR)
    else:
        nc.tensor.matmul(out, lhsT2(0), rhs2(0), start=start, stop=False)
        nc.tensor.matmul(out, lhsT2(1), rhs2(1), start=False, stop=stop)


def _build(mode: str):
    import concourse.tile as tile
    from concourse import bacc, mybir

    F16 = mybir.dt.float16
    F32 = mybir.dt.float32
    F8 = mybir.dt.float8e4
    Exp = mybir.ActivationFunctionType.Exp
    DR = mybir.MatmulPerfMode.DoubleRow
    Mult = mybir.AluOpType.mult
    Add = mybir.AluOpType.add

    nc = bacc.Bacc("TRN2", target_bir_lowering=False, debug=False,
                   num_devices=N_CORES)
    dp = nc.declare_dram_parameter
    # pre-rearranged SBUF layouts (host packs; every DMA is contiguous rows)
    m1 = dp("m1", [128, 128], F16, isOutput=False)         # c >= p
    m2 = dp("m2", [128, 2, 256], F8, isOutput=False)       # c >= 128j + p
    bq = dp("bq", [128, 4], F32, isOutput=False)
    bk = dp("bk", [128, 4], F32, isOutput=False)
    bv = dp("bv", [1, GS], F16, isOutput=False)            # 16*b_v
    wv = dp("wv", [128, 4, 2, GS], F8, isOutput=False)     # 16*w_v.T chunks
    xv = dp("xv", [128, 4, 4, 2, 512], F8, isOutput=False)  # [p,rg,jp,i,m]
    wq = dp("wq", [128, 4, 2, GS], F8, isOutput=False)
    wk = dp("wk", [128, 4, 2, GS], F8, isOutput=False)
    xq = dp("xq", [128, 4, 2, S], F8, isOutput=False)      # [p,jp,i,s]
    xk = dp("xk", [128, 4, 2, S], F8, isOutput=False)
    # exact fp16 inputs for positions 0:256 (early-query patch)
    xv16 = dp("xv16", [128, 8, 256], F16, isOutput=False)
    wv16 = dp("wv16", [128, 8, GS], F16, isOutput=False)   # 16*w_v slice, T
    xq16 = dp("xq16", [128, 8, 256], F16, isOutput=False)
    xk16 = dp("xk16", [128, 8, 256], F16, isOutput=False)
    wq16 = dp("wq16", [128, 8, GS], F16, isOutput=False)   # w_q slice, T
    wk16 = dp("wk16", [128, 8, GS], F16, isOutput=False)
    wo = dp("wo", [128, NT, D_MODEL], F16, isOutput=False)
    out = dp("partial", [D_MODEL, S], F16, isOutput=True)

    with tile.TileContext(nc) as tc:
        with tc.tile_pool(name="persist", bufs=1) as persist, \
             tc.tile_pool(name="xpool", bufs=1) as xpool, \
             tc.tile_pool(name="ppool", bufs=1) as ppool, \
             tc.tile_pool(name="psum", bufs=1, space="PSUM") as psum:

            QTs = [persist.tile([128, S], F16, name=f"qts{t}")
                   for t in range(NT)]
            KTs = [persist.tile([128, S], F16, name=f"kts{t}")
                   for t in range(NT)]
            # fp8 Vhat, key-block pairs: [:, h, i, :] = kb 2j+i, 16*V | 1/4
            # last dim padded 65->80: DoubleRow LDWEIGHTS requires the
            # pair-dim byte stride to be a multiple of 16
            Vp8 = [persist.tile([128, 8, 2, 80], F8, name=f"vp8_{j}")
                   for j in range(NKB // 2)]
            # fp16 copy for kb 0/1 (early-query patch)
            Vp16 = persist.tile([128, 8, 2, 65], F16, name="vp16")
            # ctx storage: pair t's ctx lands in QTs[t-1] (free once pair
            # t-1's S matmuls are done); only t=0 needs a dedicated tile
            ctx0 = persist.tile([128, S], F16, name="ctx0")
            ctxs = [ctx0] + QTs[:NT - 1]

            bq_sb = persist.tile([128, 4], F32, name="bq_sb")
            bk_sb = persist.tile([128, 4], F32, name="bk_sb")
            bv_row = persist.tile([1, GS], F16, name="bv_row")
            bv_bc = persist.tile([128, GS], F16, name="bv_bc")
            m1_sb = persist.tile([128, 128], F16, name="m1_sb")
            m2_sb = persist.tile([128, 2, 256], F8, name="m2_sb")

            nc.sync.dma_start(out=m1_sb[:], in_=m1[:])
            nc.sync.dma_start(out=m2_sb[:], in_=m2[:])
            nc.sync.dma_start(out=bq_sb[:], in_=bq[:])
            nc.sync.dma_start(out=bk_sb[:], in_=bk[:])
            nc.sync.dma_start(out=bv_row[:], in_=bv[:])
            nc.gpsimd.partition_broadcast(bv_bc[:], bv_row[:])

            # HAM warm-up: dense dummy matmuls gated only on the small mask
            # DMA, so the PE clock-gate reaches 8/8 while input DMAs stream.
            warm_ps = psum.tile([128, 128], F32, tag="S_e", bufs=1,
                                name="warm_ps")
            for wi in range(90):
                nc.tensor.matmul(
                    warm_ps[:, (wi % 2) * 64:(wi % 2) * 64 + 64],
                    m1_sb[:, 0:128], m1_sb[:, 0:64],
                    start=True, stop=True)

            # ones columns (denominator accumulators): 1/4 so 64*ctx stays
            # inside fp8 range pre-normalize; real-scale ctx restored by a
            # 1/16 fold in the normalize multiply (16*V * 4/den / 16).
            for j in range(NKB // 2):
                nc.vector.memset(Vp8[j][:, :, :, 64:65], 0.25)
            nc.vector.memset(Vp16[:, :, :, 64:65], 0.25)

            # prime p pools with zeros: the diagonal mask-mul multiplies a
            # stale 128-col slice of row j=1 by 0 -- uninitialized SBUF NaN
            # patterns would survive (NaN*0=NaN) and poison the AV matmul
            for pi in range(6):
                pz = ppool.tile([128, 2, W], F8, tag="p8", bufs=6,
                                name=f"pz8_{pi}")
                nc.gpsimd.memset(pz[:], 0.0)
            for pi in range(4):
                pz16 = ppool.tile([128, 2, W], F16, tag="p16", bufs=4,
                                  name=f"pz16_{pi}")
                nc.vector.memset(pz16[:], 0.0)

            ps_i = 0

            def next_ps(shape, tags):
                nonlocal ps_i
                tag = tags[ps_i % len(tags)]
                ps_i += 1
                return psum.tile(shape, F32, tag=tag, bufs=1, name=f"ps{ps_i}")

            # ---- V projection (fp8 DoubleRow over chunk pairs) ----
            # rg 0/1 (key blocks 0-7, all window-0 AV needs) run up front;
            # rg 2/3 are deferred as fillers into window (0,1) so Q/K
            # proj(0) isn't stuck behind them on the PE queue
            wv_t = persist.tile([128, 4, 2, GS], F8, name="wv_t")
            nc.sync.dma_start(out=wv_t[:], in_=wv[:])

            def vproj_rg(vsl, rg):
                for ri in range(4):
                    r = rg * 4 + ri
                    pv = next_ps([128, GS], ("S_e", "S_o"))
                    for jp in range(4):
                        _mm_pair(
                            nc, DR, pv[:],
                            lambda i, jp=jp, ri=ri: (
                                vsl[jp][:, :, ri * 128:(ri + 1) * 128]
                                if i is None else
                                vsl[jp][:, i, ri * 128:(ri + 1) * 128]),
                            lambda i, jp=jp: (
                                wv_t[:, jp, :, :] if i is None
                                else wv_t[:, jp, i, :]),
                            start=(jp == 0), stop=(jp == 3))
                    nc.vector.tensor_add(
                        Vp8[r // 2][:, :, r % 2, 0:64],
                        pv[:].rearrange("p (a b) -> p a b", a=8),
                        bv_bc[:].rearrange("p (a b) -> p a b", a=8))

            vsl_rg = {}
            for rg in range(2):
                vsl = []
                for jp in range(4):
                    s_ = xpool.tile([128, 2, 512], F8, tag="vx", bufs=8,
                                    name=f"vsl{rg}_{jp}")
                    nc.sync.dma_start(out=s_[:], in_=xv[:, rg, jp, :, :])
                    vsl.append(s_)
                vsl_rg[rg] = vsl
                vproj_rg(vsl, rg)

            # Q/K weights + resident x strips
            w_tiles = {}
            for name, dram in (("wq", wq), ("wk", wk)):
                t_ = persist.tile([128, 4, 2, GS], F8, name=f"{name}_t")
                nc.sync.dma_start(out=t_[:], in_=dram[:])
                w_tiles[name] = t_
            xq_res, xk_res = [], []
            for nm, dram, res in (("xq", xq, xq_res), ("xk", xk, xk_res)):
                for jp in range(4):
                    st = persist.tile([128, 2, S], F8, name=f"{nm}r{jp}")
                    nc.sync.dma_start(out=st[:], in_=dram[:, jp, :, :])
                    res.append(st)

            # deferred V x-slices (key blocks 8-15): consumed by fillers
            # inside window (0,1)
            for rg in range(2, 4):
                vsl = []
                for jp in range(4):
                    s_ = xpool.tile([128, 2, 512], F8, tag="vx", bufs=8,
                                    name=f"vsl{rg}_{jp}")
                    nc.sync.dma_start(out=s_[:], in_=xv[:, rg, jp, :, :])
                    vsl.append(s_)
                vsl_rg[rg] = vsl

            # fp16 patch inputs (positions 0:256), after the strips: their
            # consumers run late in window (0,0) / as fillers
            wv16_t = persist.tile([128, 8, GS], F16, name="wv16_t")
            nc.sync.dma_start(out=wv16_t[:], in_=wv16[:])
            xv16_t = persist.tile([128, 8, 256], F16, name="xv16_t")
            nc.sync.dma_start(out=xv16_t[:], in_=xv16[:])
            wq16_t = persist.tile([128, 8, GS], F16, name="wq16_t")
            wk16_t = persist.tile([128, 8, GS], F16, name="wk16_t")
            xq16_t = persist.tile([128, 8, 256], F16, name="xq16_t")
            xk16_t = persist.tile([128, 8, 256], F16, name="xk16_t")
            for t_, dram in ((wq16_t, wq16), (wk16_t, wk16),
                             (xq16_t, xq16), (xk16_t, xk16)):
                nc.sync.dma_start(out=t_[:], in_=dram[:])
            # wo last: only needed at the output projection
            wo_t = persist.tile([128, NT, D_MODEL], F16, name="wo_t")
            nc.sync.dma_start(out=wo_t[:], in_=wo[:])

            def emit_pv16(r):
                """Exact fp16 V projection for seq rows r*128:(r+1)*128."""
                pv16 = next_ps([128, GS], ("S_e", "S_o"))
                for k in range(8):
                    nc.tensor.matmul(
                        pv16[:], xv16_t[:, k, r * 128:(r + 1) * 128],
                        wv16_t[:, k, :], start=(k == 0), stop=(k == 7))
                nc.vector.tensor_add(
                    Vp16[:, :, r, 0:64],
                    pv16[:].rearrange("p (a b) -> p a b", a=8),
                    bv_bc[:].rearrange("p (a b) -> p a b", a=8))
            Q16s = [persist.tile([128, 256], F16, name=f"q16_{t}")
                    for t in range(NT)]
            K16s = [persist.tile([128, 256], F16, name=f"k16_{t}")
                    for t in range(NT)]

            def proj_group(t, wname, res, bias_sb, outs, ng):
                """One psum group (half window) of the Q/K projection for
                head pair t (fp8 DR, bias on DVE)."""
                w_t = w_tiles[wname]
                pq = next_ps([128, W], ("S_e", "S_o"))
                for jp in range(4):
                    for hf in range(2):
                        c0 = ng * W + hf * 512
                        _mm_pair(
                            nc, DR, pq[:, hf * 512:(hf + 1) * 512],
                            lambda i, jp=jp: (
                                w_t[:, jp, :, t * 128:(t + 1) * 128]
                                if i is None else
                                w_t[:, jp, i, t * 128:(t + 1) * 128]),
                            lambda i, jp=jp, c0=c0: (
                                res[jp][:, :, c0:c0 + 512]
                                if i is None else
                                res[jp][:, i, c0:c0 + 512]),
                            start=(jp == 0), stop=(jp == 3))
                nc.vector.tensor_scalar(
                    outs[t][:, ng * W:(ng + 1) * W], pq[:],
                    1.0 / 16.0, bias_sb[:, t:t + 1], Mult, Add)

            def emit_proj(t, wname, res, bias_sb, outs):
                for ng in range(2):
                    proj_group(t, wname, res, bias_sb, outs, ng)

            def emit_proj16(t, w16_t, x16_t, bias_sb, outs16):
                """Exact fp16 Q/K for positions 0:256 (early-query patch)."""
                pq16 = next_ps([128, 256], ("S_e", "S_o"))
                for k in range(8):
                    nc.tensor.matmul(
                        pq16[:], w16_t[:, k, t * 128:(t + 1) * 128],
                        x16_t[:, k, :], start=(k == 0), stop=(k == 7))
                nc.vector.tensor_scalar_add(
                    outs16[t][:], pq16[:], bias_sb[:, t:t + 1])

            def emit_attention(t, qh, fillers=None, pre_tail=None,
                               last=False):
                """Attention for heads (2t, 2t+1), q window qh.

                `fillers` are emission closures (projection psum groups for
                the next head pair) interleaved between attention pairs so
                the PE chews them during exp stalls instead of serializing
                them at window boundaries. The fp16 kb 0/1 units and the
                window normalize are returned as a `tail` closure that the
                NEXT window emits after its first pair, so the exp stream
                never drains at window boundaries. `pre_tail` is the
                previous window's such closure.
                """
                fillers = list(fillers or [])
                pre_tail_box = [pre_tail]
                kbs = [kb for kb in range(NKB)
                       if _rel_start(kb, qh, mode) < W]
                # fp16 patch: window 0, kb 0/1 (early queries)
                kb16 = [kb for kb in kbs if qh == 0 and kb < 2] \
                    if mode == "tril" else []
                kb8 = [kb for kb in kbs if kb not in kb16]
                pairs = [(kb8[2 * i], kb8[2 * i + 1])
                         for i in range(len(kb8) // 2)]
                # per 512-bank: contributing AV units in EMISSION order
                # (pairs first, then the fp16 kbs) for start/stop flags
                unit_fa = {}
                unit_rs0 = {}
                units = [("f8", pr) for pr in pairs] + \
                        [("f16", kb) for kb in kb16]
                for u in units:
                    kb0 = u[1] if u[0] == "f16" else u[1][0]
                    rs0 = _rel_start(kb0, qh, mode)
                    unit_rs0[u] = rs0
                    unit_fa[u] = (rs0 // 512) * 512
                bank_units = [[u for u in units
                               if unit_fa[u] <= bk_ * 512]
                              for bk_ in range(W // 512)]

                ctx_e = next_ps([65, W], ("ctx_e",))
                ctx_o = next_ps([65, W], ("ctx_o",))

                def do_av(ctx_ps, hi, u, p_ap):
                    fa = unit_fa[u]
                    rs0 = unit_rs0[u]
                    av_banks = list(range(fa // 512, W // 512))
                    if u[0] == "f16":
                        kb = u[1]
                        lhsT = Vp16[:, 2 * t + hi, kb, :]
                        pm = None
                    else:
                        kb = u[1][0]
                        pm = DR
                    if mode == "tril" and \
                            qh * W <= kb * 128 < (qh + 1) * W:
                        mbank = rs0 // 512
                        av_banks = [x for x in av_banks if x != mbank] \
                            + [mbank]
                    for bk_ in av_banks:
                        first = (u == bank_units[bk_][0])
                        # first-emitted unit must cover the full bank (its
                        # start=True clears has_written); its sub-rs0 head
                        # is zeroed in p8h by an explicit memset
                        a = bk_ * 512 if first else max(bk_ * 512, rs0)
                        b = (bk_ + 1) * 512
                        sp_ = (u == bank_units[bk_][-1])
                        if pm is None:
                            nc.tensor.matmul(
                                ctx_ps[:, a:b], lhsT, p_ap(a, b, None),
                                start=first, stop=sp_)
                        else:
                            _mm_pair(
                                nc, DR, ctx_ps[:, a:b],
                                lambda i, kb=kb: (
                                    Vp8[kb // 2][:, 2 * t + hi, :, 0:65]
                                    if i is None else
                                    Vp8[kb // 2][:, 2 * t + hi, i, 0:65]),
                                lambda i, a=a, b=b: p_ap(a, b, i),
                                start=first, stop=sp_)

                # fp8 DoubleRow pairs, software-pipelined one stage: pair
                # i's AV matmuls are emitted AFTER pair i+1's S/exp/mask so
                # the next S refill sits ahead of the AV chain in the PE
                # queue and the exp stream never waits behind it
                pend_av = []
                for pi_, pr in enumerate(pairs):
                    rs0 = _rel_start(pr[0], qh, mode)
                    fa = (rs0 // 512) * 512
                    diag = mode == "tril" and \
                        qh * W <= pr[0] * 128 < (qh + 1) * W
                    ps_pair = []
                    for j, kb in enumerate(pr):
                        rs = _rel_start(kb, qh, mode)
                        s_e = psum.tile([128, W], F32, tag="S_e", bufs=1,
                                        name=f"se{t}_{qh}_{kb}")
                        s_o = psum.tile([128, W], F32, tag="S_o", bufs=1,
                                        name=f"so{t}_{qh}_{kb}")
                        # bank-major, head-minor: e/o pairs row-pack
                        for bk_ in range(fa // 512, W // 512):
                            a = max(bk_ * 512, rs)
                            b = (bk_ + 1) * 512
                            for s_ps, po in ((s_e, 0), (s_o, 64)):
                                nc.tensor.matmul(
                                    s_ps[:, a:b],
                                    KTs[t][po:po + 64,
                                           kb * 128:(kb + 1) * 128],
                                    QTs[t][po:po + 64,
                                           qh * W + a:qh * W + b],
                                    start=True, stop=True)
                        ps_pair.append((s_e, s_o))
                    p8h = [ppool.tile([128, 2, W], F8, tag="p8", bufs=6,
                                      name=f"p8_{t}_{qh}_{pr[0]}_{h_}")
                           for h_ in range(2)]
                    u = ("f8", pr)
                    # first-emitted unit of a bank reads the full bank:
                    # zero the [bank, rs0) head its exps never write
                    for bk_ in range(fa // 512, W // 512):
                        if u == bank_units[bk_][0] and bk_ * 512 < rs0:
                            for hi in range(2):
                                nc.gpsimd.memset(
                                    p8h[hi][:, :, bk_ * 512:rs0], 0.0)
                    # exp order e(kb0), o(kb0), e(kb1), o(kb1): next kb's
                    # S matmuls refill the freed psum during the odd exp
                    for j, kb in enumerate(pr):
                        rs = _rel_start(kb, qh, mode)
                        for hi in range(2):
                            nc.scalar.activation(
                                p8h[hi][:, j, rs:W],
                                ps_pair[j][hi][:, rs:W],
                                Exp, scale=1.0 / 8.0)
                    if diag:
                        # one narrow mask per head: zeroes kb1's [rs0,rs1)
                        # and the triangular band of both kbs
                        for hi in range(2):
                            nc.vector.tensor_mul(
                                p8h[hi][:, :, rs0:rs0 + 256],
                                p8h[hi][:, :, rs0:rs0 + 256],
                                m2_sb[:])

                    def av_pair(u=u, p8h=p8h):
                        for hi in range(2):
                            do_av((ctx_e, ctx_o)[hi], hi, u,
                                  lambda a, b, i=None, hi=hi: (
                                      p8h[hi][:, :, a:b] if i is None
                                      else p8h[hi][:, i, a:b]))
                    # previous window's tail, then filler, AFTER this
                    # pair's S refill (so they run during the exps) and
                    # BEFORE the previous pair's AV chain
                    if pi_ >= 1 and pre_tail_box[0] is not None:
                        pt = pre_tail_box[0]
                        pre_tail_box[0] = None
                        pt()
                    elif pi_ >= 1 and fillers:
                        fillers.pop(0)()
                    if pend_av:
                        pend_av.pop(0)()
                    pend_av.append(av_pair)
                if pre_tail_box[0] is not None:
                    pt = pre_tail_box[0]
                    pre_tail_box[0] = None
                    pt()
                while pend_av:
                    pend_av.pop(0)()
                while fillers:
                    fillers.pop(0)()

                # fp16 patch units LAST (kb 0, 1 of window 0): exact fp16
                # scores/V for early queries; their AV matmuls overwrite
                # the still-unwritten [0, 256) ctx columns (has_written=0)
                # and accumulate elsewhere. Together with the window
                # normalize they form the `tail`, emitted inside the NEXT
                # window.
                def emit_tail():
                    for kb in kb16:
                        rs = _rel_start(kb, qh, mode)
                        s_e = psum.tile([128, W], F32, tag="S_e", bufs=1,
                                        name=f"se{t}_{qh}_{kb}")
                        s_o = psum.tile([128, W], F32, tag="S_o", bufs=1,
                                        name=f"so{t}_{qh}_{kb}")
                        for bk_ in range(W // 512):
                            a = max(bk_ * 512, rs)
                            b = (bk_ + 1) * 512
                            for s_ps, po in ((s_e, 0), (s_o, 64)):
                                nc.tensor.matmul(
                                    s_ps[:, a:b],
                                    KTs[t][po:po + 64,
                                           kb * 128:(kb + 1) * 128],
                                    QTs[t][po:po + 64,
                                           qh * W + a:qh * W + b],
                                    start=True, stop=True)
                        # overwrite the (q<256, k<256) block with exact
                        # fp16 scores: early queries' entire support
                        for s_ps, po in ((s_e, 0), (s_o, 64)):
                            nc.tensor.matmul(
                                s_ps[:, rs:256],
                                K16s[t][po:po + 64,
                                        kb * 128:(kb + 1) * 128],
                                Q16s[t][po:po + 64, rs:256],
                                start=True, stop=True)
                        p16 = ppool.tile([128, 2, W], F16, tag="p16",
                                         bufs=4, name=f"p16_{t}_{kb}")
                        for hi, s_ps in ((0, s_e), (1, s_o)):
                            nc.scalar.activation(p16[:, hi, rs:W],
                                                 s_ps[:, rs:W],
                                                 Exp, scale=1.0 / 8.0)
                        # triangular mask over the 128-col diagonal band
                        for hi in range(2):
                            nc.vector.tensor_mul(
                                p16[:, hi, rs:rs + 128],
                                p16[:, hi, rs:rs + 128], m1_sb[:])
                        for hi in range(2):
                            u16 = ("f16", kb)
                            do_av((ctx_e, ctx_o)[hi], hi, u16,
                                  lambda a, b, i=None, p16=p16, hi=hi:
                                  p16[:, hi, a:b])

                    # normalize both heads for this window
                    for hi, ctx_ps in ((0, ctx_e), (1, ctx_o)):
                        po = hi * 64
                        # denominator row (1 partition) -> SBUF on DVE
                        d1 = ppool.tile([1, W], F32, tag="d1", bufs=1,
                                        name=f"d1_{t}_{qh}_{hi}")
                        nc.vector.tensor_copy(d1[:], ctx_ps[64:65, :])
                        cr = ppool.tile([64, W], F32, tag="cr", bufs=1,
                                        name=f"cr{t}_{qh}_{hi}")
                        nc.vector.tensor_copy(cr[:], ctx_ps[0:64, :])
                        # psum free from here; chain runs off SBUF
                        d2 = ppool.tile([128, 8], F32, tag="d2", bufs=1,
                                        name=f"d2_{t}_{qh}_{hi}")
                        nc.sync.dma_start(out=d2[:], in_=d1[:])
                        d3 = ppool.tile([128, 8], F32, tag="d3", bufs=1,
                                        name=f"d3_{t}_{qh}_{hi}")
                        nc.vector.reciprocal(d3[:], d2[:])
                        d4 = ppool.tile([1, W], F32, tag="d4", bufs=1,
                                        name=f"d4_{t}_{qh}_{hi}")
                        nc.sync.dma_start(out=d4[:], in_=d3[:])
                        bc = ppool.tile([64, W], F32, tag="bc", bufs=1,
                                        name=f"bc{t}_{qh}_{hi}")
                        nc.gpsimd.partition_broadcast(bc[:], d4[:])
                        # cr holds 16*num, bc = 4/den -> cr*bc = 64*ctx
                        nc.vector.scalar_tensor_tensor(
                            ctxs[t][po:po + 64, qh * W:(qh + 1) * W],
                            cr[:], 1.0 / 64.0, bc[:],
                            op0=Mult, op1=Mult)

                if last:
                    emit_tail()
                    return None
                return emit_tail

            # ---- projections for t=0, then attention with interleaved
            # proj(t+1) emission at window boundaries ----
            def emit_outproj_half(mo, half, tags):
                """Output projection for rows mo*128:(mo+1)*128, seq half
                half*1024:(half+1)*1024 (2 psum groups + copies + DMA).
                Half 0 only needs the qh=0 windows of every ctx, so it can
                fill the last attention window; half 1 is the tail."""
                ot = xpool.tile([128, W], F16, tag="os", bufs=3,
                                name=f"ot{mo}_{half}")
                for nn in range(2):
                    n = 2 * half + nn
                    pp = next_ps([128, 512], tags)
                    for tt in range(NT):
                        nc.tensor.matmul(
                            pp[:], wo_t[:, tt, mo * 128:(mo + 1) * 128],
                            ctxs[tt][:, n * 512:(n + 1) * 512],
                            start=(tt == 0), stop=(tt == NT - 1))
                    # alternate evacuation engine to halve each queue chain
                    if nn % 2 == 0:
                        nc.scalar.copy(ot[:, nn * 512:(nn + 1) * 512], pp[:])
                    else:
                        nc.vector.tensor_copy(
                            ot[:, nn * 512:(nn + 1) * 512], pp[:])
                nc.sync.dma_start(
                    out=out[mo * 128:(mo + 1) * 128,
                            half * W:(half + 1) * W],
                    in_=ot[:])

            emit_proj(0, "wq", xq_res, bq_sb, QTs)
            emit_proj(0, "wk", xk_res, bk_sb, KTs)
            ptail = None
            for t in range(NT):
                f0 = []
                if t + 1 < NT:
                    for ng in range(2):
                        f0.append(lambda ng=ng, t1=t + 1: proj_group(
                            t1, "wq", xq_res, bq_sb, QTs, ng))
                if mode == "tril" and t == 0:
                    # patch projections late: their DMAs follow the strips
                    f0.append(lambda: emit_pv16(0))
                    f0.append(lambda: emit_pv16(1))
                    f0.append(lambda: emit_proj16(0, wq16_t, xq16_t,
                                                  bq_sb, Q16s))
                    f0.append(lambda: emit_proj16(0, wk16_t, xk16_t,
                                                  bk_sb, K16s))
                if mode == "full" and t == 0:
                    # full mode: window (0,0) already reads kb 8-15
                    f0 = [lambda: vproj_rg(vsl_rg[2], 2),
                          lambda: vproj_rg(vsl_rg[3], 3)] + f0
                ptail = emit_attention(t, 0, fillers=f0, pre_tail=ptail)
                f1 = []
                if mode == "tril" and t == 0:
                    # deferred V projection for key blocks 8-15 (needed
                    # from this window's later pairs on)
                    f1.append(lambda: vproj_rg(vsl_rg[2], 2))
                    f1.append(lambda: vproj_rg(vsl_rg[3], 3))
                if t + 1 < NT:
                    for ng in range(2):
                        f1.append(lambda ng=ng, t1=t + 1: proj_group(
                            t1, "wk", xk_res, bk_sb, KTs, ng))
                    if mode == "tril":
                        f1.append(lambda t1=t + 1: emit_proj16(
                            t1, wq16_t, xq16_t, bq_sb, Q16s))
                        f1.append(lambda t1=t + 1: emit_proj16(
                            t1, wk16_t, xk16_t, bk_sb, K16s))
                ptail = emit_attention(t, 1, fillers=f1, pre_tail=ptail,
                                       last=(t == NT - 1))

            # ---- output projection (fp16) ----
            for mo in range(8):
                emit_outproj_half(mo, 0, ("S_e", "S_o", "ctx_e", "ctx_o"))
                emit_outproj_half(mo, 1, ("S_e", "S_o", "ctx_e", "ctx_o"))

    nc.compile()
    return nc


def _get_program(mode: str):
    if mode not in _prog_cache:
        _install_neff_cache()
        _prog_cache[mode] = _build(mode)
    return _prog_cache[mode]


def _numpy_fallback(query, key, value, w_q, b_q, w_k, b_k, w_v, b_v,
                    w_o, b_o, mask):
    def split_heads(x):
        b, s, _ = x.shape
        return x.reshape(b, s, N_HEADS, D_K).transpose(0, 2, 1, 3)

    Q = split_heads(query @ w_q.T + b_q)
    K = split_heads(key @ w_k.T + b_k)
    V = split_heads(value @ w_v.T + b_v)
    out = np.empty((B, N_HEADS, S, D_K), np.float32)
    m2 = np.asarray(mask).reshape(mask.shape[-2], mask.shape[-1])
    for b in range(B):
        for h in range(N_HEADS):
            s = (Q[b, h] @ K[b, h].T) / np.sqrt(np.float32(D_K))
            s = np.where(m2, s, np.finfo(np.float32).min)
            s = s - s.max(axis=-1, keepdims=True)
            e = np.exp(s)
            out[b, h] = (e / e.sum(axis=-1, keepdims=True)) @ V[b, h]
    ctx = out.transpose(0, 2, 1, 3).reshape(B, S, D_MODEL)
    return (ctx @ w_o.T + b_o).astype(np.float32)


def kernel(query, key, value, w_q, b_q, w_k, b_k, w_v, b_v, w_o, b_o, mask):
    import ml_dtypes

    f8 = ml_dtypes.float8_e4m3
    query = np.asarray(query, np.float32)
    key = np.asarray(key, np.float32)
    value = np.asarray(value, np.float32)
    w_q, w_k = np.asarray(w_q, np.float32), np.asarray(w_k, np.float32)
    w_v, w_o = np.asarray(w_v, np.float32), np.asarray(w_o, np.float32)
    b_q, b_k = np.asarray(b_q, np.float32), np.asarray(b_k, np.float32)
    b_v, b_o = np.asarray(b_v, np.float32), np.asarray(b_o, np.float32)

    m2 = np.asarray(mask).reshape(mask.shape[-2], mask.shape[-1]).astype(bool)
    if m2.all():
        mode = "full"
    elif np.array_equal(m2, np.tril(np.ones((S, S), bool))):
        mode = "tril"
    else:
        return _numpy_fallback(query, key, value, w_q, b_q, w_k, b_k,
                               w_v, b_v, w_o, b_o, mask)

    from concourse.bass_utils import run_bass_kernel_spmd

    nc = _get_program(mode)

    # constant mask tiles
    pp_, cc = np.arange(128)[:, None], np.arange(128)[None, :]
    m1h = np.ascontiguousarray((cc >= pp_).astype(np.float16))  # [128,128]
    c256 = np.arange(256)[None, None, :]
    jj = np.arange(2)[None, :, None]
    m2h = np.ascontiguousarray(
        (c256 >= (128 * jj + np.arange(128)[:, None, None])
         ).astype(f8))                                          # [128,2,256]

    def strips(a):   # [1024, X] -> [128, 4, 2, X]
        return np.ascontiguousarray(
            a.reshape(4, 2, 128, a.shape[1]).transpose(2, 0, 1, 3))

    def chunks8(a):  # [1024, X] -> [128, 8, X]
        return np.ascontiguousarray(
            a.reshape(8, 128, a.shape[1]).transpose(1, 0, 2))

    in_maps = []
    for c in range(N_CORES):
        b, g = c // 2, c % 2
        sl = slice(g * GS, (g + 1) * GS)
        xvt = np.ascontiguousarray(value[b].T).astype(f8)      # [1024, 2048]
        xv_l = np.ascontiguousarray(
            xvt.reshape(4, 2, 128, 4, 512).transpose(2, 3, 0, 1, 4))
        in_maps.append({
            "m1": m1h,
            "m2": m2h,
            "bq": np.ascontiguousarray(b_q[sl].reshape(4, 128).T),
            "bk": np.ascontiguousarray(b_k[sl].reshape(4, 128).T),
            "bv": np.ascontiguousarray(
                16.0 * b_v[sl][None, :]).astype(np.float16),
            "wv": strips((16.0 * w_v[sl, :].T).astype(f8)),
            "xv": xv_l,
            "wq": strips((16.0 * w_q[sl, :].T).astype(f8)),
            "wk": strips((16.0 * w_k[sl, :].T).astype(f8)),
            "xq": strips(np.ascontiguousarray(query[b].T).astype(f8)),
            "xk": strips(np.ascontiguousarray(key[b].T).astype(f8)),
            "xv16": chunks8(
                np.ascontiguousarray(value[b, 0:256].T).astype(np.float16)),
            "wv16": chunks8((16.0 * w_v[sl, :].T).astype(np.float16)),
            "xq16": chunks8(
                np.ascontiguousarray(query[b, 0:256].T).astype(np.float16)),
            "xk16": chunks8(
                np.ascontiguousarray(key[b, 0:256].T).astype(np.float16)),
            "wq16": chunks8((w_q[sl, :].T).astype(np.float16)),
            "wk16": chunks8((w_k[sl, :].T).astype(np.float16)),
            "wo": np.ascontiguousarray(
                (w_o[:, sl].T).astype(np.float16).reshape(
                    4, 128, D_MODEL).transpose(1, 0, 2)),
        })

    global _last_in_maps, _last_res
    _last_in_maps = in_maps
    res = run_bass_kernel_spmd(nc, in_maps, list(range(N_CORES)), trace=False)
    _last_res = res

    out = np.empty((B, S, D_MODEL), np.float32)
    for b in range(B):
        p0 = res.results[2 * b]["partial"].astype(np.float32)
        p1 = res.results[2 * b + 1]["partial"].astype(np.float32)
        out[b] = (p0 + p1).T + b_o
    return out
